# revision 20
# baseline (speedup 1.0000x reference)
"""Trainium2 Bass kernel for BaseLayerWithLoRA (dense_mlp).

Computes out = x @ W.T + b + (x @ lora_A) @ lora_B for
x:[4,2048,4096] W:[4096,4096] b:[4096] lora_A:[4096,16] lora_B:[16,4096].

Sharding across 8 NeuronCores: 4-way data-parallel over rows of x
(B*S = 8192 -> 2048 rows/core) x 2-way tensor-parallel over the output
dim O (4096 -> 2048 cols/core). lora_A is replicated; W, b, lora_B are
column-sharded. No collectives needed; the host gathers the 8 output
shards.

Device kernel (per core, all bf16 inputs, fp32 accumulate):
  - W.T shard [4096, 2048] resident in SBUF as [128, 32, 2048]
  - per 128-row m-tile of x.T: one [128, 32, 128] SBUF tile feeds
    (a) the LoRA pass (xA).T = lora_A.T @ x.T  -> PSUM [16, 128]
    (b) the base matmuls: psum[128m, 512o] += xT_k.T @ wT_k over 32 k
  - the LoRA delta and bias are folded into the same PSUM accumulation
    with one extra matmul: lhsT = [(xA).T ; ones] (17 x 128),
    rhs = [lora_B ; b] (17 x 512) -> adds xA@B + 1*b.
"""

import os
import sys

import numpy as np

try:
    import concourse.bass as bass  # noqa: F401
except ImportError:  # pragma: no cover
    for p in ("/opt/trn_rl_repo", "/root/.axon_site/_ro/trn_rl_repo"):
        if os.path.isdir(p) and p not in sys.path:
            sys.path.insert(0, p)
    import concourse.bass as bass  # noqa: F401

import ml_dtypes
from contextlib import ExitStack

import concourse.tile as tile
from concourse import bacc, mybir
from concourse.bass import ts
from concourse.bass_utils import run_bass_kernel_spmd
from concourse.masks import make_identity

BF16 = ml_dtypes.bfloat16

# Problem shapes (hardcoded per contract).
B, S, I, O, R = 4, 2048, 4096, 4096, 16
M_TOT = B * S  # 8192 rows
DP, TP = 4, 2  # core grid: 4 data-parallel x 2 tensor-parallel
N_CORES = DP * TP

P = 128  # partitions

# Stash of the most recent BassKernelResults (for test harness introspection).
LAST_RESULTS = None


def build_nc(M, ON, KI, n_cores=N_CORES, repeat=1, xbufs=5, k_outer=False):
    """Build the single-core SPMD program.

    M: rows per core, ON: output cols per core, KI: contraction dim.
    repeat>1 wraps the whole body in an on-device loop (for timing).
    """
    KT = KI // P          # k-chunks of 128
    NO = min(512, ON)     # psum free width
    MT = M // P           # m-tiles
    OC = ON // NO         # o-chunks
    RB = R + 1            # lora rank + bias row

    nc = bacc.Bacc("TRN2", target_bir_lowering=False, debug=False,
                   num_devices=n_cores)

    xT = nc.dram_tensor("xT", [KI, M], mybir.dt.bfloat16, kind="ExternalInput").ap()
    wT = nc.dram_tensor("wT", [KI, ON], mybir.dt.bfloat16, kind="ExternalInput").ap()
    aT = nc.dram_tensor("aT", [KI, R], mybir.dt.bfloat16, kind="ExternalInput").ap()
    bb = nc.dram_tensor("bb", [RB, ON], mybir.dt.bfloat16, kind="ExternalInput").ap()
    out = nc.dram_tensor("out", [M, ON], mybir.dt.float32, kind="ExternalOutput").ap()

    with tile.TileContext(nc) as tc, ExitStack() as ctx:
        wpool = ctx.enter_context(tc.tile_pool(name="wpool", bufs=OC))
        cpool = ctx.enter_context(tc.tile_pool(name="cpool", bufs=1))
        xpool = ctx.enter_context(tc.tile_pool(name="xpool", bufs=xbufs))
        xapool = ctx.enter_context(tc.tile_pool(name="xapool", bufs=3))
        opool = ctx.enter_context(tc.tile_pool(name="opool", bufs=6))
        pspool = ctx.enter_context(tc.tile_pool(name="pspool", bufs=6, space="PSUM"))
        papool = ctx.enter_context(tc.tile_pool(name="papool", bufs=2, space="PSUM"))

        rep_ctx = tc.For_i(0, repeat, 1) if repeat > 1 else None
        if rep_ctx is not None:
            rep_ctx.__enter__()

        xT3 = xT.rearrange("(ko ki) m -> ki ko m", ki=P)
        wT3 = wT.rearrange("(ko ki) o -> ki ko o", ki=P)

        # First x tile + LoRA constants land before the weight chunks so the
        # PE can start immediately; W is loaded as OC column chunks, each
        # unlocking one whole oc accumulation group.
        xsb0 = xpool.tile([P, KT, P], mybir.dt.bfloat16, name="xsb0", tag="xtile")
        nc.sync.dma_start(out=xsb0[:], in_=xT3[:, :, ts(0, P)])
        asb = cpool.tile([P, KT, R], mybir.dt.bfloat16, name="asb")
        nc.sync.dma_start(out=asb[:], in_=aT.rearrange("(ko ki) r -> ki ko r", ki=P))
        bbsb = cpool.tile([RB, ON], mybir.dt.bfloat16, name="bbsb")
        nc.sync.dma_start(out=bbsb[:], in_=bb[:])

        wtiles = []
        for g in range(OC):
            wsb = wpool.tile([P, KT, NO], mybir.dt.bfloat16, name=f"wsb{g}",
                             tag="wchunk")
            nc.sync.dma_start(out=wsb[:], in_=wT3[:, :, ts(g, NO)])
            wtiles.append(wsb)

        xtiles = {0: xsb0}
        xatiles = {}

        def pxa_pass(mt):
            """LoRA first stage: (x @ A).T for m-tile mt (A-stationary,
            one PSUM bank, bank-consecutive MMs) -> [R+1, 128] bf16."""
            xsb = xtiles[mt]
            pxa = papool.tile([R, P], mybir.dt.float32, name=f"pxa{mt}",
                              tag="pxa")
            for k in range(KT):
                nc.tensor.matmul(pxa[:], asb[:, k, :], xsb[:, k, :],
                                 start=(k == 0), stop=(k == KT - 1))
            xasb = xapool.tile([RB, P], mybir.dt.bfloat16, name=f"xasb{mt}",
                               tag="xat")
            # Row R is a constant 1.0 (bias row); memset the whole tile then
            # overwrite rows 0..R-1 (memset start-partition must be 0).
            nc.any.memset(xasb[:], 1.0)
            nc.scalar.copy(xasb[:R, :], pxa[:])
            return xasb

        # Run the first PRE pxa passes up front: they depend only on x tiles
        # (1 MB each), giving the PE work while the 16.8 MB of W streams in.
        PRE = min(4, MT)
        for mt in range(1, PRE):
            xn = xpool.tile([P, KT, P], mybir.dt.bfloat16, name=f"xsb{mt}",
                            tag="xtile")
            nc.sync.dma_start(out=xn[:], in_=xT3[:, :, ts(mt, P)])
            xtiles[mt] = xn
        for mt in range(PRE):
            xatiles[mt] = pxa_pass(mt)

        for mt in range(MT):
            xsb = xtiles[mt]
            # Prefetch the next x tile not yet in flight.
            nxt = mt + PRE
            if nxt < MT:
                xn = xpool.tile([P, KT, P], mybir.dt.bfloat16,
                                name=f"xsb{nxt}", tag="xtile")
                nc.sync.dma_start(out=xn[:], in_=xT3[:, :, ts(nxt, P)])
                xtiles[nxt] = xn
            if mt not in xatiles:
                xatiles[mt] = pxa_pass(mt)
            xasb = xatiles.pop(mt)

            pss = [pspool.tile([P, NO], mybir.dt.float32, name=f"ps{mt}_{oc}",
                               tag="ps") for oc in range(OC)]
            if k_outer:
                for k in range(KT):
                    for oc in range(OC):
                        nc.tensor.matmul(pss[oc][:], xsb[:, k, :],
                                         wtiles[oc][:, k, :],
                                         start=(k == 0), stop=False)
            else:
                for oc in range(OC):
                    for k in range(KT):
                        nc.tensor.matmul(pss[oc][:], xsb[:, k, :],
                                         wtiles[oc][:, k, :],
                                         start=(k == 0), stop=False)
            for oc in range(OC):
                # LoRA second stage + bias, fused into the same accumulation.
                nc.tensor.matmul(pss[oc][:], xasb[:], bbsb[:, ts(oc, NO)],
                                 start=False, stop=True)
                osb = opool.tile([P, NO], mybir.dt.float32, name=f"osb{mt}_{oc}",
                                 tag="osb")
                nc.vector.tensor_copy(osb[:], pss[oc][:])
                nc.sync.dma_start(out=out[ts(mt, P), ts(oc, NO)], in_=osb[:])

        if rep_ctx is not None:
            rep_ctx.__exit__(None, None, None)

    nc.compile()
    return nc


_NC_CACHE = {}


def _get_nc():
    key = "full"
    if key not in _NC_CACHE:
        _NC_CACHE[key] = build_nc(M_TOT // DP, O // TP, I)
    return _NC_CACHE[key]


def kernel(x, W, b, lora_A, lora_B):
    global LAST_RESULTS
    M = M_TOT // DP
    ON = O // TP

    xf = np.asarray(x, dtype=np.float32).reshape(M_TOT, I)
    x_bf = xf.astype(BF16)
    W = np.asarray(W, dtype=np.float32)
    b = np.asarray(b, dtype=np.float32)
    lora_A = np.asarray(lora_A, dtype=np.float32)
    lora_B = np.asarray(lora_B, dtype=np.float32)

    xT_shards = [np.ascontiguousarray(x_bf[dp * M:(dp + 1) * M, :].T)
                 for dp in range(DP)]
    wT_shards = [np.ascontiguousarray(
        W[tp * ON:(tp + 1) * ON, :].astype(BF16).T) for tp in range(TP)]
    bb_shards = [np.concatenate(
        [lora_B[:, tp * ON:(tp + 1) * ON],
         b[None, tp * ON:(tp + 1) * ON]], axis=0).astype(BF16)
        for tp in range(TP)]
    aT_rep = np.ascontiguousarray(lora_A.astype(BF16))

    in_maps = []
    for c in range(N_CORES):
        dp, tp = divmod(c, TP)
        in_maps.append({
            "xT": xT_shards[dp],
            "wT": wT_shards[tp],
            "aT": aT_rep,
            "bb": bb_shards[tp],
        })

    nc = _get_nc()
    res = run_bass_kernel_spmd(nc, in_maps, list(range(N_CORES)))
    LAST_RESULTS = res

    out_full = np.empty((M_TOT, O), dtype=np.float32)
    for c in range(N_CORES):
        dp, tp = divmod(c, TP)
        out_full[dp * M:(dp + 1) * M, tp * ON:(tp + 1) * ON] = res.results[c]["out"]
    return out_full.reshape(B, S, O)
